# revision 7
# baseline (speedup 1.0000x reference)
"""Trainium2 Bass kernel for nn_BinnedLoss (tent-weighted 128-bin chi2 loss).

Self-contained: builds an 8-core SPMD Bass program, shards the N=16.7M
sample axis across cores, computes per-core partial histograms via a
two-level (q8 x r16) one-hot + TensorE matmul accumulation, all-reduces
the 128-bin histograms, and finishes the chi2 on every core.

kernel(**inputs) -> np.float32 scalar (shape ()).
"""
import os
import sys

sys.path.insert(0, "/opt/trn_rl_repo")
import numpy as np

N = 16777216
NCORES = 8
BINS = 128
P = 128
NSH = N // NCORES            # samples per core
FTOT_FULL = NSH // P         # 16384 free columns per core per array
NB = 256                     # columns (sample groups) per pipeline tick
NPS = 3                      # PSUM accumulator banks per array
MAGIC = 8388608.0            # 2^23 round-to-nearest trick


def _patches(mybir, tile):
    from concourse.vector_clock import ScopedClock

    def _patched(self, tick_clock, wait_clock):
        drain_inst = self.nc.sync.drain()
        wait_clock.add_sem_waits(
            drain_inst.ins, ScopedClock({None: tick_clock.global_clock})
        )
        si = drain_inst.ins.sync_info
        if si is not None and si.on_wait and len(si.on_wait) > 1:
            waits = list(si.on_wait)
            drain_inst.ins.sync_info = mybir.SyncInfo(
                on_wait=[waits[0]], on_update=list(si.on_update)
            )
            for w in waits[1:]:
                nop = self.nc.sync.nop()
                nop.ins.sync_info = mybir.SyncInfo(on_wait=[w], on_update=[])
        self.nc.all_engine_barrier()
        assert self.sems is not None
        popped = self.nc._tile_sem_poison_stack.pop()
        assert popped is self._sem_poison
        self.nc.clear_and_free_semaphores(list(self.sems.allocated().values()))
        self.nc.all_engine_barrier()

    tile.TileContext._drain_and_barrier = _patched


def _split_sync_waits(nc, mybir):
    """This walrus build allows <=1 sem-wait per instruction; hoist extras
    onto same-engine NOPs inserted just before the instruction."""
    counter = [0]
    for f in nc.m.functions:
        for bb in f.blocks:
            out = []
            dirty = False
            for inst in bb.instructions:
                si = inst.sync_info
                if si is not None and si.on_wait and len(si.on_wait) > 1:
                    waits = list(si.on_wait)
                    for w in waits[:-1]:
                        counter[0] += 1
                        nop = mybir.InstNoOp(
                            name=f"WSPLIT-{counter[0]}", ins=[], outs=[]
                        )
                        nop.engine = inst.engine
                        nop.sync_info = mybir.SyncInfo(on_wait=[w], on_update=[])
                        nc.register_instruction(nop, overwrite=True)
                        out.append(nop)
                    inst.sync_info = mybir.SyncInfo(
                        on_wait=[waits[-1]], on_update=list(si.on_update)
                    )
                    dirty = True
                out.append(inst)
            if dirty:
                bb.instructions = out


def build(ftot=FTOT_FULL, ncores=NCORES, repeat=1, nb=NB):
    import concourse.bass as bass
    import concourse.mybir as mybir
    from concourse import tile

    _patches(mybir, tile)
    DT = mybir.dt
    AL = mybir.AluOpType
    ACT = mybir.ActivationFunctionType
    F32 = DT.float32
    BF16 = DT.bfloat16
    core_ids = list(range(ncores))

    nc = bass.Bass()
    sim_ext = nc.declare_dram_parameter("sim", [P, ftot], F32, isOutput=False)
    exp_ext = nc.declare_dram_parameter("exp", [P, ftot], F32, isOutput=False)
    w_ext = nc.declare_dram_parameter("w", [P, ftot], F32, isOutput=False)
    out_ext = nc.declare_dram_parameter("out", [1, 1], F32, isOutput=True)

    with tile.TileContext(nc) as tc:
        with (
            tc.tile_pool(name="const", bufs=1) as cpool,
            tc.tile_pool(name="dram", bufs=1, space="DRAM") as dram,
            tc.tile_pool(name="psum", bufs=1, space="PSUM") as psum,
        ):
            cc_a_in = dram.tile([1, 2], F32, name="cc_a_in")
            cc_a_out = dram.tile([1, 2], F32, name="cc_a_out")
            cc_h_in = dram.tile([1, 512], F32, name="cc_h_in")
            cc_h_out = dram.tile([1, 512], F32, name="cc_h_out")

            # constants
            io8_i = cpool.tile([P, 8], DT.int16, name="io8_i")
            io16_i = cpool.tile([P, 16], DT.int16, name="io16_i")
            nc.gpsimd.iota(io8_i[:], [[1, 8]], channel_multiplier=0)
            nc.gpsimd.iota(io16_i[:], [[1, 16]], channel_multiplier=0)
            io8 = cpool.tile([P, 8], BF16, name="io8")
            io16 = cpool.tile([P, 16], BF16, name="io16")
            nc.vector.tensor_copy(io8[:], io8_i[:])
            nc.vector.tensor_copy(io16[:], io16_i[:])
            zU = cpool.tile([P, 32], BF16, name="zU")
            zV = cpool.tile([P, 16], BF16, name="zV")
            nc.vector.memset(zU[:], 0.0)
            nc.vector.memset(zV[:], 0.0)
            ones1 = cpool.tile([1, P], F32, name="ones1")
            nc.vector.memset(ones1[:], 1.0)

            # scalar staging: sc = [mn, step, inv, bias0, mn+step, delta]
            sc = cpool.tile([1, 6], F32, name="sc")
            bc = cpool.tile([P, 6], F32, name="bc")

            banks = [
                psum.tile([32, 16], F32, name=f"psb{k}", tag=f"psb{k}")
                for k in range(2 * NPS)
            ]
            bcps = psum.tile([P, 6], F32, name="bcps", tag="bcps")
            hist_s = cpool.tile([1, BINS], F32, name="hist_s")
            hist_e = cpool.tile([1, BINS], F32, name="hist_e")

            for rep in range(repeat):
                # ---------------- Phase A: global min/max ----------------
                with tc.tile_pool(name=f"pa{rep}", bufs=3) as pa:
                    CW = min(2048, ftot)
                    rmin = pa.tile([P, 1], F32, name="rmin", bufs=1)
                    rmax = pa.tile([P, 1], F32, name="rmax", bufs=1)
                    first = True
                    for arr in (sim_ext, exp_ext):
                        for c0 in range(0, ftot, CW):
                            ch = pa.tile([P, CW], F32, name="ch", tag="ch")
                            nc.sync.dma_start(ch[:], arr[:, c0:c0 + CW])
                            tmin = pa.tile([P, 1], F32, name="tmin", tag="tmin")
                            tmax = pa.tile([P, 1], F32, name="tmax", tag="tmax")
                            nc.vector.tensor_reduce(
                                tmin[:], ch[:], mybir.AxisListType.X, AL.min)
                            nc.vector.tensor_reduce(
                                tmax[:], ch[:], mybir.AxisListType.X, AL.max)
                            if first:
                                nc.vector.tensor_copy(rmin[:], tmin[:])
                                nc.vector.tensor_copy(rmax[:], tmax[:])
                                first = False
                            else:
                                nc.vector.tensor_tensor(rmin[:], rmin[:], tmin[:], AL.min)
                                nc.vector.tensor_tensor(rmax[:], rmax[:], tmax[:], AL.max)
                    # partition collapse via SBUF->SBUF DMA reshape
                    pm = pa.tile([1, 2 * P], F32, name="pm", bufs=1)
                    nc.gpsimd.dma_start(pm[0:1, 0:P], rmax[:, 0:1])
                    nc.gpsimd.dma_start(pm[0:1, P:2 * P], rmin[:, 0:1])
                    pk = pa.tile([1, 2], F32, name="pk", bufs=1)
                    nc.vector.tensor_reduce(
                        pk[0:1, 0:1], pm[0:1, 0:P], mybir.AxisListType.X, AL.max)
                    nc.vector.tensor_reduce(
                        pk[0:1, 1:2], pm[0:1, P:2 * P], mybir.AxisListType.X, AL.min)
                    nc.vector.tensor_scalar_mul(pk[0:1, 1:2], pk[0:1, 1:2], -1.0)
                    nc.gpsimd.dma_start(cc_a_in[:], pk[:])
                    nc.gpsimd.collective_compute(
                        "AllReduce", AL.max, replica_groups=[core_ids],
                        ins=[cc_a_in.opt()], outs=[cc_a_out.opt()],
                    )
                    ga = pa.tile([1, 2], F32, name="ga", bufs=1)
                    nc.gpsimd.dma_start(ga[:], cc_a_out[:])
                    # scalars: ga = [mx, -mn]
                    nc.vector.tensor_scalar_mul(sc[0:1, 0:1], ga[0:1, 1:2], -1.0)  # mn
                    d_t = pa.tile([1, 1], F32, name="d_t", bufs=1)
                    nc.vector.tensor_tensor(d_t[:], ga[0:1, 0:1], sc[0:1, 0:1], AL.subtract)
                    nc.vector.tensor_scalar_mul(
                        sc[0:1, 1:2], d_t[:], float(np.float32(1.0) / np.float32(127.0)))
                    nc.vector.reciprocal(sc[0:1, 2:3], sc[0:1, 1:2])     # inv
                    nc.vector.scalar_tensor_tensor(
                        sc[0:1, 3:4], sc[0:1, 0:1], -1.0, sc[0:1, 2:3],
                        AL.mult, AL.mult)                                 # -mn*inv
                    nc.vector.tensor_tensor(
                        sc[0:1, 4:5], sc[0:1, 0:1], sc[0:1, 1:2], AL.add)  # mn+step
                    nc.vector.tensor_scalar_mul(sc[0:1, 5:6], d_t[:], 0.0078125)  # delta
                    nc.tensor.matmul(bcps[:], ones1[:], sc[0:1, :],
                                     start=True, stop=True)
                    nc.vector.tensor_copy(bc[:], bcps[:])

                # ---------------- Phase B: binning ----------------
                for k in range(2 * NPS):
                    nc.tensor.matmul(banks[k][:], zU[:], zV[:],
                                     start=True, stop=False, skip_group_check=True)

                for ai, (arr, weighted) in enumerate(((sim_ext, True), (exp_ext, False))):
                    abanks = banks[ai * NPS:(ai + 1) * NPS]

                    def load(pipe, iv):
                        xt = pipe.intermediate_tile([P, nb], F32, name="xt")
                        nc.sync.dma_start(xt[:], arr[:, bass.ds(iv, nb)])
                        if weighted:
                            wt = pipe.intermediate_tile([P, nb], F32, name="wt")
                            nc.sync.dma_start(wt[:], w_ext[:, bass.ds(iv, nb)])
                            return (xt, wt)
                        return (xt,)

                    def compute(pipe, iv, tiles):
                        x = tiles[0]
                        wgt = tiles[1] if weighted else None
                        t = lambda nm: pipe.intermediate_tile([P, nb], F32, name=nm)
                        u = t("u")
                        nc.scalar.activation(u[:], x[:], ACT.Identity,
                                             bias=bc[:, 3:4], scale=bc[:, 2:3])
                        kc = t("kc")
                        nc.vector.tensor_scalar(kc[:], u[:], MAGIC, -MAGIC, AL.add, AL.add)
                        gt = t("gt")
                        nc.vector.tensor_tensor(gt[:], kc[:], u[:], AL.is_gt)
                        nc.vector.tensor_tensor(kc[:], kc[:], gt[:], AL.subtract)
                        nc.vector.tensor_scalar(kc[:], kc[:], 0.0, 126.0, AL.max, AL.min)
                        hk = t("hk")
                        nc.scalar.activation(hk[:], kc[:], ACT.Identity,
                                             bias=bc[:, 0:1], scale=bc[:, 1:2])
                        hk1 = t("hk1")
                        nc.scalar.activation(hk1[:], kc[:], ACT.Identity,
                                             bias=bc[:, 4:5], scale=bc[:, 1:2])
                        m1 = t("m1")
                        nc.vector.tensor_tensor(m1[:], x[:], hk[:], AL.is_ge)
                        m2 = t("m2")
                        nc.vector.tensor_tensor(m2[:], x[:], hk1[:], AL.is_lt)
                        nc.vector.tensor_tensor(m1[:], m1[:], m2[:], AL.mult)  # in_iv
                        mp = t("mp")
                        nc.vector.tensor_scalar(mp[:], kc[:], 125.5, None, AL.is_lt)
                        nc.vector.tensor_tensor(mp[:], mp[:], m1[:], AL.mult)
                        mm = t("mm")
                        nc.vector.tensor_scalar(mm[:], kc[:], 0.5, None, AL.is_gt)
                        nc.vector.tensor_tensor(mm[:], mm[:], m1[:], AL.mult)
                        cp = t("cp")
                        nc.vector.tensor_tensor(cp[:], x[:], hk[:], AL.subtract)
                        if weighted:
                            nc.vector.tensor_tensor(cp[:], cp[:], wgt[:], AL.mult)
                        nc.vector.tensor_tensor(cp[:], cp[:], mp[:], AL.mult)
                        cm = t("cm")
                        nc.vector.tensor_tensor(cm[:], hk1[:], x[:], AL.subtract)
                        if weighted:
                            nc.vector.tensor_tensor(cm[:], cm[:], wgt[:], AL.mult)
                        nc.vector.tensor_tensor(cm[:], cm[:], mm[:], AL.mult)
                        b = lambda nm: pipe.intermediate_tile([P, nb], BF16, name=nm)
                        cph, cpl, cmh, cml = b("cph"), b("cpl"), b("cmh"), b("cml")
                        nc.vector.tensor_copy(cph[:], cp[:])
                        nc.vector.tensor_tensor(cpl[:], cp[:], cph[:], AL.subtract)
                        nc.vector.tensor_copy(cmh[:], cm[:])
                        nc.vector.tensor_tensor(cml[:], cm[:], cmh[:], AL.subtract)
                        t16 = t("t16")
                        nc.vector.tensor_scalar_mul(t16[:], kc[:], 0.0625)
                        qf = t("qf")
                        nc.vector.tensor_scalar(qf[:], t16[:], MAGIC, -MAGIC, AL.add, AL.add)
                        gt2 = t("gt2")
                        nc.vector.tensor_tensor(gt2[:], qf[:], t16[:], AL.is_gt)
                        nc.vector.tensor_tensor(qf[:], qf[:], gt2[:], AL.subtract)
                        r = t("r")
                        nc.vector.scalar_tensor_tensor(
                            r[:], qf[:], -16.0, kc[:], AL.mult, AL.add)
                        oq = pipe.intermediate_tile([P, nb, 8], BF16, name="oq")
                        nc.vector.tensor_tensor(
                            oq[:], io8[:, :].unsqueeze(1).broadcast_to([P, nb, 8]),
                            qf[:].unsqueeze(2).broadcast_to([P, nb, 8]), AL.is_equal)
                        U = pipe.intermediate_tile([P, nb, 32], BF16, name="U")
                        for si_, s in enumerate((cph, cpl, cmh, cml)):
                            nc.vector.tensor_tensor(
                                U[:, :, si_ * 8:(si_ + 1) * 8], oq[:],
                                s[:].unsqueeze(2).broadcast_to([P, nb, 8]), AL.mult)
                        V = pipe.intermediate_tile([P, nb, 16], BF16, name="V")
                        nc.vector.tensor_tensor(
                            V[:], io16[:, :].unsqueeze(1).broadcast_to([P, nb, 16]),
                            r[:].unsqueeze(2).broadcast_to([P, nb, 16]), AL.is_equal)
                        return (U, V)

                    def pe(pipe, iv, UV):
                        U, V = UV
                        for j in range(nb):
                            nc.tensor.matmul(
                                abanks[j % NPS][:], U[:, j, :], V[:, j, :],
                                start=False, stop=False, skip_group_check=True)

                    tc.For_i_pipelined(
                        [load, compute, pe], 0, ftot, step=nb,
                        unroll=2, name=f"bin{rep}_{ai}")

                # ---------------- Phase C: reduce + chi2 ----------------
                with tc.tile_pool(name=f"pc{rep}", bufs=1) as pc:
                    pkH = pc.tile([1, 512], F32, name="pkH")
                    for ai in range(2):
                        abanks = banks[ai * NPS:(ai + 1) * NPS]
                        H = pc.tile([32, 16], F32, name=f"H{ai}")
                        nc.vector.tensor_copy(H[:], abanks[0][:])
                        for k in range(1, NPS):
                            nc.vector.tensor_tensor(H[:], H[:], abanks[k][:], AL.add)
                        Hf = pc.tile([1, 512], F32, name=f"Hf{ai}")
                        nc.gpsimd.dma_start(Hf[:], H[:])
                        a0 = ai * 256
                        nc.vector.tensor_tensor(
                            pkH[0:1, a0:a0 + 128], Hf[0:1, 0:128],
                            Hf[0:1, 128:256], AL.add)
                        nc.vector.tensor_tensor(
                            pkH[0:1, a0 + 128:a0 + 256], Hf[0:1, 256:384],
                            Hf[0:1, 384:512], AL.add)
                    nc.gpsimd.dma_start(cc_h_in[:], pkH[:])
                    nc.gpsimd.collective_compute(
                        "AllReduce", AL.add, replica_groups=[core_ids],
                        ins=[cc_h_in.opt()], outs=[cc_h_out.opt()],
                    )
                    gh = pc.tile([1, 512], F32, name="gh")
                    nc.gpsimd.dma_start(gh[:], cc_h_out[:])
                    for ai, hist in enumerate((hist_s, hist_e)):
                        a0 = ai * 256
                        nc.vector.memset(hist[:], 0.0)
                        nc.vector.tensor_tensor(
                            hist[0:1, 1:127], gh[0:1, a0:a0 + 126],
                            gh[0:1, a0 + 129:a0 + 255], AL.add)
                        ssum = pc.tile([1, 1], F32, name=f"ssum{ai}")
                        nc.vector.tensor_reduce(
                            ssum[:], hist[:], mybir.AxisListType.X, AL.add)
                        nc.vector.tensor_tensor(ssum[:], ssum[:], sc[0:1, 5:6], AL.mult)
                        nc.vector.reciprocal(ssum[:], ssum[:])
                        nc.vector.tensor_scalar(
                            hist[:], hist[:], ssum[0:1, 0:1], None, AL.mult)
                    dif = pc.tile([1, BINS], F32, name="dif")
                    nc.vector.tensor_tensor(dif[:], hist_s[:], hist_e[:], AL.subtract)
                    nc.vector.tensor_tensor(dif[:], dif[:], dif[:], AL.mult)
                    chi = pc.tile([1, 1], F32, name="chi")
                    nc.vector.tensor_reduce(
                        chi[:], dif[:], mybir.AxisListType.X, AL.add)
                    nc.gpsimd.dma_start(out_ext[:], chi[:])

    _split_sync_waits(nc, __import__("concourse.mybir", fromlist=["x"]))
    return nc


_CACHE = {}


def _get_nc(repeat):
    key = repeat
    if key not in _CACHE:
        _CACHE[key] = build(repeat=repeat)
    return _CACHE[key]


def kernel(**inputs):
    sim = np.ascontiguousarray(inputs["sim_observable"], dtype=np.float32)
    exp = np.ascontiguousarray(inputs["exp_observable"], dtype=np.float32)
    w = np.ascontiguousarray(inputs["weights"], dtype=np.float32)
    assert sim.shape == (N,) and exp.shape == (N,) and w.shape == (N,)

    from concourse.bass_utils import run_bass_kernel_spmd

    repeat = int(os.environ.get("BASS_HIST_REPEAT", "1"))
    nc = _get_nc(repeat)
    sim_s = sim.reshape(NCORES, P, FTOT_FULL)
    exp_s = exp.reshape(NCORES, P, FTOT_FULL)
    w_s = w.reshape(NCORES, P, FTOT_FULL)
    in_maps = [
        {"sim": sim_s[c], "exp": exp_s[c], "w": w_s[c]} for c in range(NCORES)
    ]
    res = run_bass_kernel_spmd(nc, in_maps, list(range(NCORES)))
    val = res.results[0]["out"][0, 0]
    return np.asarray(val, dtype=np.float32).reshape(())


# revision 11
# speedup vs baseline: 8.8205x; 8.8205x over previous
"""Trainium2 Bass kernel for nn_BinnedLoss (tent-weighted 128-bin chi2 loss).

Self-contained: builds an 8-core SPMD Bass program, shards the N=16.7M
sample axis across cores, computes per-core partial histograms via a
two-level (q8 x r16) one-hot + TensorE matmul accumulation, all-reduces
the 128-bin histograms, and finishes the chi2 on every core.

kernel(**inputs) -> np.float32 scalar (shape ()).
"""
import os
import sys

sys.path.insert(0, "/opt/trn_rl_repo")
import numpy as np

N = 16777216
NCORES = 8
BINS = 128
P = 128
NSH = N // NCORES            # samples per core
FTOT_FULL = NSH // P         # 16384 free columns per core per array
NB = 512                     # columns (sample groups) per pipeline tick
NPS = 3                      # PSUM accumulator banks per array
GG = 16                      # groups fused per matmul (diag-block scheme)
QW, RW = 4, 32               # kc = QW-level q in [0,4) x r in [0,32)
MAGIC = 8388608.0            # 2^23 round-to-nearest trick


def _patches(mybir, tile):
    from concourse.vector_clock import ScopedClock

    def _patched(self, tick_clock, wait_clock):
        drain_inst = self.nc.sync.drain()
        wait_clock.add_sem_waits(
            drain_inst.ins, ScopedClock({None: tick_clock.global_clock})
        )
        si = drain_inst.ins.sync_info
        if si is not None and si.on_wait and len(si.on_wait) > 1:
            waits = list(si.on_wait)
            drain_inst.ins.sync_info = mybir.SyncInfo(
                on_wait=[waits[0]], on_update=list(si.on_update)
            )
            for w in waits[1:]:
                nop = self.nc.sync.nop()
                nop.ins.sync_info = mybir.SyncInfo(on_wait=[w], on_update=[])
        self.nc.all_engine_barrier()
        assert self.sems is not None
        popped = self.nc._tile_sem_poison_stack.pop()
        assert popped is self._sem_poison
        self.nc.clear_and_free_semaphores(list(self.sems.allocated().values()))
        self.nc.all_engine_barrier()

    tile.TileContext._drain_and_barrier = _patched


def _split_sync_waits(nc, mybir):
    """This walrus build allows <=1 sem-wait per instruction; hoist extras
    onto same-engine NOPs inserted just before the instruction."""
    counter = [0]
    for f in nc.m.functions:
        for bb in f.blocks:
            out = []
            dirty = False
            for inst in bb.instructions:
                si = inst.sync_info
                if si is not None and si.on_wait and len(si.on_wait) > 1:
                    waits = list(si.on_wait)
                    for w in waits[:-1]:
                        counter[0] += 1
                        nop = mybir.InstNoOp(
                            name=f"WSPLIT-{counter[0]}", ins=[], outs=[]
                        )
                        nop.engine = inst.engine
                        nop.sync_info = mybir.SyncInfo(on_wait=[w], on_update=[])
                        nc.register_instruction(nop, overwrite=True)
                        out.append(nop)
                    inst.sync_info = mybir.SyncInfo(
                        on_wait=[waits[-1]], on_update=list(si.on_update)
                    )
                    dirty = True
                out.append(inst)
            if dirty:
                bb.instructions = out


def build(ftot=FTOT_FULL, ncores=NCORES, repeat=1, nb=NB, variant='full'):
    import concourse.bass as bass
    import concourse.mybir as mybir
    from concourse import tile

    _patches(mybir, tile)
    DT = mybir.dt
    AL = mybir.AluOpType
    ACT = mybir.ActivationFunctionType
    F32 = DT.float32
    BF16 = DT.bfloat16
    core_ids = list(range(ncores))

    nc = bass.Bass()
    sim_ext = nc.declare_dram_parameter("sim", [P, ftot], F32, isOutput=False)
    exp_ext = nc.declare_dram_parameter("exp", [P, ftot], F32, isOutput=False)
    w_ext = nc.declare_dram_parameter("w", [P, ftot], F32, isOutput=False)
    out_ext = nc.declare_dram_parameter("out", [1, 1], F32, isOutput=True)

    with tile.TileContext(nc) as tc:
        with (
            tc.tile_pool(name="const", bufs=1) as cpool,
            tc.tile_pool(name="dram", bufs=1, space="DRAM") as dram,
            tc.tile_pool(name="psum", bufs=1, space="PSUM") as psum,
        ):
            cc_a_in = dram.tile([1, 2], F32, name="cc_a_in")
            cc_a_out = dram.tile([1, 2], F32, name="cc_a_out")
            cc_h_in = dram.tile([1, 512], F32, name="cc_h_in")
            cc_h_out = dram.tile([1, 512], F32, name="cc_h_out")

            # constants
            ioq_i = cpool.tile([P, QW], DT.int16, name="ioq_i")
            ior_i = cpool.tile([P, RW], DT.int16, name="ior_i")
            nc.gpsimd.iota(ioq_i[:], [[1, QW]], channel_multiplier=0)
            nc.gpsimd.iota(ior_i[:], [[1, RW]], channel_multiplier=0)
            ioq = cpool.tile([P, QW], BF16, name="ioq")
            ior = cpool.tile([P, RW], BF16, name="ior")
            nc.vector.tensor_copy(ioq[:], ioq_i[:])
            nc.vector.tensor_copy(ior[:], ior_i[:])
            zU = cpool.tile([P, P], BF16, name="zU")
            zV = cpool.tile([P, GG * RW], BF16, name="zV")
            nc.vector.memset(zU[:], 0.0)
            nc.vector.memset(zV[:], 0.0)
            ones1 = cpool.tile([1, P], F32, name="ones1")
            nc.vector.memset(ones1[:], 1.0)

            # scalar staging: sc = [mn, step, inv, bias0, mn+step, delta]
            sc = cpool.tile([1, 6], F32, name="sc")
            bc = cpool.tile([P, 6], F32, name="bc")

            banks = [
                psum.tile([P, GG * RW], F32, name=f"psb{k}", tag=f"psb{k}")
                for k in range(2 * NPS)
            ]
            bcps = psum.tile([P, 6], F32, name="bcps", tag="bcps")
            hist_s = cpool.tile([1, BINS], F32, name="hist_s")
            hist_e = cpool.tile([1, BINS], F32, name="hist_e")

            for rep in range(repeat):
                # ---------------- Phase A: global min/max ----------------
                with tc.tile_pool(name=f"pa{rep}", bufs=3) as pa:
                    CW = min(2048, ftot)
                    rmin = pa.tile([P, 1], F32, name="rmin", bufs=1)
                    rmax = pa.tile([P, 1], F32, name="rmax", bufs=1)
                    first = True
                    for arr in (sim_ext, exp_ext):
                        for c0 in range(0, ftot, CW):
                            ch = pa.tile([P, CW], F32, name="ch", tag="ch")
                            nc.sync.dma_start(ch[:], arr[:, c0:c0 + CW])
                            tmin = pa.tile([P, 1], F32, name="tmin", tag="tmin")
                            tmax = pa.tile([P, 1], F32, name="tmax", tag="tmax")
                            nc.vector.tensor_reduce(
                                tmin[:], ch[:], mybir.AxisListType.X, AL.min)
                            nc.vector.tensor_reduce(
                                tmax[:], ch[:], mybir.AxisListType.X, AL.max)
                            if first:
                                nc.vector.tensor_copy(rmin[:], tmin[:])
                                nc.vector.tensor_copy(rmax[:], tmax[:])
                                first = False
                            else:
                                nc.vector.tensor_tensor(rmin[:], rmin[:], tmin[:], AL.min)
                                nc.vector.tensor_tensor(rmax[:], rmax[:], tmax[:], AL.max)
                    # partition collapse via SBUF->SBUF DMA reshape
                    pm = pa.tile([1, 2 * P], F32, name="pm", bufs=1)
                    nc.gpsimd.dma_start(pm[0:1, 0:P], rmax[:, 0:1])
                    nc.gpsimd.dma_start(pm[0:1, P:2 * P], rmin[:, 0:1])
                    pk = pa.tile([1, 2], F32, name="pk", bufs=1)
                    nc.vector.tensor_reduce(
                        pk[0:1, 0:1], pm[0:1, 0:P], mybir.AxisListType.X, AL.max)
                    nc.vector.tensor_reduce(
                        pk[0:1, 1:2], pm[0:1, P:2 * P], mybir.AxisListType.X, AL.min)
                    nc.vector.tensor_scalar_mul(pk[0:1, 1:2], pk[0:1, 1:2], -1.0)
                    nc.gpsimd.dma_start(cc_a_in[:], pk[:])
                    nc.gpsimd.collective_compute(
                        "AllReduce", AL.max, replica_groups=[core_ids],
                        ins=[cc_a_in.opt()], outs=[cc_a_out.opt()],
                    )
                    ga = pa.tile([1, 2], F32, name="ga", bufs=1)
                    nc.gpsimd.dma_start(ga[:], cc_a_out[:])
                    # scalars: ga = [mx, -mn]
                    nc.vector.tensor_scalar_mul(sc[0:1, 0:1], ga[0:1, 1:2], -1.0)  # mn
                    d_t = pa.tile([1, 1], F32, name="d_t", bufs=1)
                    nc.vector.tensor_tensor(d_t[:], ga[0:1, 0:1], sc[0:1, 0:1], AL.subtract)
                    nc.vector.tensor_scalar_mul(
                        sc[0:1, 1:2], d_t[:], float(np.float32(1.0) / np.float32(127.0)))
                    nc.vector.reciprocal(sc[0:1, 2:3], sc[0:1, 1:2])     # inv
                    nc.vector.scalar_tensor_tensor(
                        sc[0:1, 3:4], sc[0:1, 0:1], -1.0, sc[0:1, 2:3],
                        AL.mult, AL.mult)                                 # -mn*inv
                    nc.vector.tensor_tensor(
                        sc[0:1, 4:5], sc[0:1, 0:1], sc[0:1, 1:2], AL.add)  # mn+step
                    nc.vector.tensor_scalar_mul(sc[0:1, 5:6], d_t[:], 0.0078125)  # delta
                    nc.tensor.matmul(bcps[:], ones1[:], sc[0:1, :],
                                     start=True, stop=True)
                    nc.vector.tensor_copy(bc[:], bcps[:])

                # ---------------- Phase B: binning ----------------
                for k in range(2 * NPS):
                    nc.tensor.matmul(banks[k][:], zU[:], zV[:],
                                     start=True, stop=False, skip_group_check=True)

                for ai, (arr, weighted) in enumerate(((sim_ext, True), (exp_ext, False))):
                    abanks = banks[ai * NPS:(ai + 1) * NPS]

                    def load(pipe, iv):
                        xt = pipe.intermediate_tile([P, nb], F32, name="xt")
                        nc.sync.dma_start(xt[:], arr[:, bass.ds(iv, nb)])
                        if weighted:
                            wt = pipe.intermediate_tile([P, nb], F32, name="wt")
                            nc.sync.dma_start(wt[:], w_ext[:, bass.ds(iv, nb)])
                            return (xt, wt)
                        return (xt,)

                    def compute(pipe, iv, tiles):
                        x = tiles[0]
                        wgt = tiles[1] if weighted else None
                        t = lambda nm: pipe.intermediate_tile([P, nb], F32, name=nm)
                        u = t("u")
                        nc.scalar.activation(u[:], x[:], ACT.Identity,
                                             bias=bc[:, 3:4], scale=bc[:, 2:3])
                        kc = t("kc")
                        nc.vector.tensor_scalar(kc[:], u[:], MAGIC, -MAGIC, AL.add, AL.add)
                        gt = t("gt")
                        nc.vector.tensor_tensor(gt[:], kc[:], u[:], AL.is_gt)
                        nc.vector.tensor_tensor(kc[:], kc[:], gt[:], AL.subtract)
                        nc.vector.tensor_scalar(kc[:], kc[:], 0.0, 126.0, AL.max, AL.min)
                        hk = t("hk")
                        nc.scalar.activation(hk[:], kc[:], ACT.Identity,
                                             bias=bc[:, 0:1], scale=bc[:, 1:2])
                        hk1 = t("hk1")
                        nc.scalar.activation(hk1[:], kc[:], ACT.Identity,
                                             bias=bc[:, 4:5], scale=bc[:, 1:2])
                        m1 = t("m1")
                        nc.vector.tensor_tensor(m1[:], x[:], hk[:], AL.is_ge)
                        m2 = t("m2")
                        nc.vector.tensor_tensor(m2[:], x[:], hk1[:], AL.is_lt)
                        nc.vector.tensor_tensor(m1[:], m1[:], m2[:], AL.mult)  # in_iv
                        mp = t("mp")
                        nc.vector.scalar_tensor_tensor(
                            mp[:], kc[:], 125.5, m1[:], AL.is_lt, AL.mult)
                        mm = t("mm")
                        nc.vector.scalar_tensor_tensor(
                            mm[:], kc[:], 0.5, m1[:], AL.is_gt, AL.mult)
                        cp = t("cp")
                        nc.vector.tensor_tensor(cp[:], x[:], hk[:], AL.subtract)
                        if weighted:
                            nc.vector.tensor_tensor(cp[:], cp[:], wgt[:], AL.mult)
                        cm = t("cm")
                        nc.vector.tensor_tensor(cm[:], hk1[:], x[:], AL.subtract)
                        if weighted:
                            nc.vector.tensor_tensor(cm[:], cm[:], wgt[:], AL.mult)
                        b = lambda nm: pipe.intermediate_tile([P, nb], BF16, name=nm)
                        cph, cmh = b("cph"), b("cmh")
                        nc.vector.tensor_tensor(cph[:], cp[:], mp[:], AL.mult)
                        nc.vector.tensor_tensor(cmh[:], cm[:], mm[:], AL.mult)
                        t32 = t("t32")
                        nc.vector.tensor_scalar_mul(t32[:], kc[:], 1.0 / RW)
                        qf = t("qf")
                        nc.vector.tensor_scalar(qf[:], t32[:], MAGIC, -MAGIC, AL.add, AL.add)
                        gt2 = t("gt2")
                        nc.vector.tensor_tensor(gt2[:], qf[:], t32[:], AL.is_gt)
                        nc.vector.tensor_tensor(qf[:], qf[:], gt2[:], AL.subtract)
                        r = t("r")
                        nc.vector.scalar_tensor_tensor(
                            r[:], qf[:], -float(RW), kc[:], AL.mult, AL.add)
                        oq = pipe.intermediate_tile([P, nb, QW], BF16, name="oq")
                        nc.vector.tensor_tensor(
                            oq[:], ioq[:, :].unsqueeze(1).broadcast_to([P, nb, QW]),
                            qf[:].unsqueeze(2).broadcast_to([P, nb, QW]), AL.is_equal)
                        U = pipe.intermediate_tile([P, nb, 2 * QW], BF16, name="U")
                        nc.vector.tensor_tensor(
                            U[:, :, 0:QW], oq[:],
                            cph[:].unsqueeze(2).broadcast_to([P, nb, QW]), AL.mult)
                        nc.vector.tensor_tensor(
                            U[:, :, QW:2 * QW], oq[:],
                            cmh[:].unsqueeze(2).broadcast_to([P, nb, QW]), AL.mult)
                        V = pipe.intermediate_tile([P, nb, RW], BF16, name="V")
                        nc.vector.tensor_tensor(
                            V[:], ior[:, :].unsqueeze(1).broadcast_to([P, nb, RW]),
                            r[:].unsqueeze(2).broadcast_to([P, nb, RW]), AL.is_equal)
                        return (U, V)

                    def pe(pipe, iv, UV):
                        U, V = UV
                        npair = nb // GG
                        if variant == 'nope':
                            npair = 1
                        for j in range(npair):
                            if variant == 'pezero' and j > 0:
                                nc.tensor.matmul(
                                    abanks[j % NPS][:], zU[:], zV[:],
                                    start=False, stop=False, skip_group_check=True)
                                continue
                            nc.tensor.matmul(
                                abanks[j % NPS][:],
                                U[:, j * GG:(j + 1) * GG, :],
                                V[:, j * GG:(j + 1) * GG, :],
                                start=False, stop=False, skip_group_check=True)

                    tc.For_i_pipelined(
                        [load, compute, pe], 0, ftot, step=nb,
                        unroll=2, name=f"bin{rep}_{ai}")

                # ---------------- Phase C: reduce + chi2 ----------------
                with tc.tile_pool(name=f"pc{rep}", bufs=1) as pc:
                    pkH = pc.tile([1, 512], F32, name="pkH")
                    for ai in range(2):
                        abanks = banks[ai * NPS:(ai + 1) * NPS]
                        Hs = pc.tile([P, GG * RW], F32, name=f"Hs{ai}")
                        nc.vector.tensor_copy(Hs[:], abanks[0][:])
                        for k in range(1, NPS):
                            nc.vector.tensor_tensor(Hs[:], Hs[:], abanks[k][:], AL.add)
                        flat = pc.tile([1, GG * 256], F32, name=f"flat{ai}")
                        for g in range(GG):
                            nc.gpsimd.dma_start(
                                flat[0:1, g * 256:(g + 1) * 256],
                                Hs[g * 8:(g + 1) * 8, g * RW:(g + 1) * RW])
                        a0 = ai * 256
                        fv = flat[0:1, :].rearrange("a (g j) -> a j g", g=GG)
                        nc.vector.tensor_reduce(
                            pkH[0:1, a0:a0 + 128], fv[:, 0:128, :],
                            mybir.AxisListType.X, AL.add)
                        nc.vector.tensor_reduce(
                            pkH[0:1, a0 + 128:a0 + 256], fv[:, 128:256, :],
                            mybir.AxisListType.X, AL.add)
                    nc.gpsimd.dma_start(cc_h_in[:], pkH[:])
                    nc.gpsimd.collective_compute(
                        "AllReduce", AL.add, replica_groups=[core_ids],
                        ins=[cc_h_in.opt()], outs=[cc_h_out.opt()],
                    )
                    gh = pc.tile([1, 512], F32, name="gh")
                    nc.gpsimd.dma_start(gh[:], cc_h_out[:])
                    for ai, hist in enumerate((hist_s, hist_e)):
                        a0 = ai * 256
                        nc.vector.memset(hist[:], 0.0)
                        nc.vector.tensor_tensor(
                            hist[0:1, 1:127], gh[0:1, a0:a0 + 126],
                            gh[0:1, a0 + 129:a0 + 255], AL.add)
                        ssum = pc.tile([1, 1], F32, name=f"ssum{ai}")
                        nc.vector.tensor_reduce(
                            ssum[:], hist[:], mybir.AxisListType.X, AL.add)
                        nc.vector.tensor_tensor(ssum[:], ssum[:], sc[0:1, 5:6], AL.mult)
                        nc.vector.reciprocal(ssum[:], ssum[:])
                        nc.vector.tensor_scalar(
                            hist[:], hist[:], ssum[0:1, 0:1], None, AL.mult)
                    dif = pc.tile([1, BINS], F32, name="dif")
                    nc.vector.tensor_tensor(dif[:], hist_s[:], hist_e[:], AL.subtract)
                    nc.vector.tensor_tensor(dif[:], dif[:], dif[:], AL.mult)
                    chi = pc.tile([1, 1], F32, name="chi")
                    nc.vector.tensor_reduce(
                        chi[:], dif[:], mybir.AxisListType.X, AL.add)
                    nc.gpsimd.dma_start(out_ext[:], chi[:])

    _split_sync_waits(nc, __import__("concourse.mybir", fromlist=["x"]))
    return nc


_CACHE = {}


def _get_nc(repeat):
    variant = os.environ.get("BASS_HIST_VARIANT", "full")
    key = (repeat, variant)
    if key not in _CACHE:
        _CACHE[key] = build(repeat=repeat, variant=variant)
    return _CACHE[key]


def kernel(**inputs):
    sim = np.ascontiguousarray(inputs["sim_observable"], dtype=np.float32)
    exp = np.ascontiguousarray(inputs["exp_observable"], dtype=np.float32)
    w = np.ascontiguousarray(inputs["weights"], dtype=np.float32)
    assert sim.shape == (N,) and exp.shape == (N,) and w.shape == (N,)

    from concourse.bass_utils import run_bass_kernel_spmd

    repeat = int(os.environ.get("BASS_HIST_REPEAT", "1"))
    nc = _get_nc(repeat)
    sim_s = sim.reshape(NCORES, P, FTOT_FULL)
    exp_s = exp.reshape(NCORES, P, FTOT_FULL)
    w_s = w.reshape(NCORES, P, FTOT_FULL)
    in_maps = [
        {"sim": sim_s[c], "exp": exp_s[c], "w": w_s[c]} for c in range(NCORES)
    ]
    res = run_bass_kernel_spmd(nc, in_maps, list(range(NCORES)))
    val = res.results[0]["out"][0, 0]
    return np.asarray(val, dtype=np.float32).reshape(())
